# revision 1
# baseline (speedup 1.0000x reference)
"""Trainium2 Bass kernel for sparse conv-transpose (gather-GEMM-scatter) + BatchNorm.

Strategy (8 NeuronCores, SPMD):
  - Shard output rows across cores (50000 rows/core). Host groups the
    2.7M (k, m) pairs by (core, k, occurrence-rank) so every scatter call
    has unique destination rows (occurrence layering makes CCE-add RMW safe).
  - Per core: indirect-DMA gather of feats rows -> PE transpose (via
    identity matmul) -> 128x128 GEMM with W[k] -> indirect-DMA scatter
    with compute_op=add into the core's raw output slice in DRAM.
  - BN stats (sum/sumsq per channel) computed on-device at the end of
    launch 1; host combines the 8 partial stats; launch 2 applies
    y = raw*scale + shift on-device.
"""

import sys

import numpy as np

sys.path.insert(0, "/opt/trn_rl_repo")

import os
import time

import concourse.bacc as bacc
import concourse.tile as tile
from concourse import bass, mybir
from concourse.bass import IndirectOffsetOnAxis
from concourse.bass_utils import run_bass_kernel_spmd

P = 128
N_CORES = 8
LAST_EXEC_NS = []  # exec_time_ns per launch (when NTFF tracing is available)
LAST_WALL_S = []   # wall seconds per launch (incl. PJRT transfer)
BLK = 4096          # pairs per staging block
CPB = BLK // P      # chunks per block (32)
def _garb(SH):
    # pad garbage region so SH+GARB is a multiple of P
    return ((-SH) % P) + P
EPS = 1e-5

f32 = mybir.dt.float32
i32 = mybir.dt.int32


def _host_prep(in_maps, out_maps, n_in, n_out):
    """Build per-core gather/scatter index streams, uniform across cores."""
    K3, M = in_maps.shape
    SH = n_out // N_CORES
    GARB = _garb(SH)
    k_all = np.repeat(np.arange(K3, dtype=np.int64), M)
    im = np.asarray(in_maps, dtype=np.int64).reshape(-1)
    om = np.asarray(out_maps, dtype=np.int64).reshape(-1)
    core = om // SH
    oml = om - core * SH

    # occurrence rank within (core, k, local-row): pairs sharing a dest row
    # within one k go to different layers -> unique dests per scatter call
    order = np.lexsort((oml, k_all, core))
    sc, sk, so, si = core[order], k_all[order], oml[order], im[order]
    new_grp = np.r_[True, (sc[1:] != sc[:-1]) | (sk[1:] != sk[:-1]) | (so[1:] != so[:-1])]
    starts = np.flatnonzero(new_grp)
    gid = np.cumsum(new_grp) - 1
    occ = np.arange(len(order)) - starts[gid]
    occ_max = int(occ.max()) + 1

    counts = np.zeros((N_CORES, K3, occ_max), np.int64)
    np.add.at(counts, (sc, sk, occ), 1)
    padded = ((counts.max(axis=0) + P - 1) // P) * P     # [K3, occ_max], shared

    # reorder pairs to (core, k, occ) grouping
    order2 = np.lexsort((occ, sk, sc))
    c2, k2, o2, i2 = sc[order2], sk[order2], so[order2], si[order2]

    gidx, sidx = [], []
    chunk_k = None
    group_bounds = None
    core_starts = np.searchsorted(c2, np.arange(N_CORES + 1))
    for c in range(N_CORES):
        s0, s1 = core_starts[c], core_starts[c + 1]
        cc_im, cc_om = i2[s0:s1], o2[s0:s1]
        cc_cnt = counts[c]
        gl, sl, ckl, cb = [], [], [], [0]
        pos = 0
        garb = 0
        for kk in range(K3):
            for rr in range(occ_max):
                n = int(cc_cnt[kk, rr])
                pn = int(padded[kk, rr])
                if pn == 0:
                    continue
                npad = pn - n
                gl.append(cc_im[pos:pos + n])
                sl.append(cc_om[pos:pos + n])
                pos += n
                if npad:
                    gl.append(np.full(npad, n_in, np.int64))
                    sl.append(SH + (np.arange(garb, garb + npad) % GARB))
                    garb += npad
                if c == 0:
                    ckl.append(np.full(pn // P, kk, np.int64))
                    cb.append(cb[-1] + pn // P)
        gidx.append(np.concatenate(gl))
        sidx.append(np.concatenate(sl))
        if c == 0:
            chunk_k = np.concatenate(ckl)
            group_bounds = cb

    TOT = len(gidx[0])
    NB = (TOT + BLK - 1) // BLK
    fill = NB * BLK - TOT
    if fill:
        for c in range(N_CORES):
            gidx[c] = np.concatenate([gidx[c], np.full(fill, n_in, np.int64)])
            sidx[c] = np.concatenate([sidx[c], SH + (np.arange(fill) % GARB)])
        chunk_k = np.concatenate([chunk_k, np.zeros(fill // P, np.int64)])
        group_bounds = group_bounds + [group_bounds[-1] + fill // P]

    nchunks = NB * CPB
    bounds = set(group_bounds)
    calls = []
    cur = 0
    for ch in range(1, nchunks + 1):
        if ch in bounds or ch % CPB == 0:
            calls.append((cur, ch))
            cur = ch

    def to_sb(a):
        # pair t=b*BLK+c*128+p lives at sbuf[p, b*CPB+c]
        return np.ascontiguousarray(
            a.astype(np.int32).reshape(NB * CPB, P).T)

    gidx = np.stack([to_sb(g) for g in gidx])
    sidx = np.stack([to_sb(s) for s in sidx])
    return dict(SH=SH, K3=K3, NB=NB, chunk_k=chunk_k, calls=calls,
                gidx=gidx, sidx=sidx)


def _build_launch1(n_in1, SH, K3, NB, chunk_k, calls):
    nc = bacc.Bacc("TRN2", target_bir_lowering=False, debug=False,
                   num_devices=N_CORES)
    feats = nc.dram_tensor("feats", [n_in1, P], f32, kind="ExternalInput")
    wcat = nc.dram_tensor("wcat", [P, K3 * P], f32, kind="ExternalInput")
    ident = nc.dram_tensor("ident", [P, P], f32, kind="ExternalInput")
    gidx_d = nc.dram_tensor("gidx", [P, NB * CPB], i32, kind="ExternalInput")
    sidx_d = nc.dram_tensor("sidx", [P, NB * CPB], i32, kind="ExternalInput")
    GARB = _garb(SH)
    raw = nc.dram_tensor("raw", [SH + GARB, P], f32, kind="ExternalOutput")
    stats = nc.dram_tensor("stats", [1, 2 * P], f32, kind="ExternalOutput")

    n_rows = SH + GARB
    assert n_rows % P == 0
    ntiles = n_rows // P
    # stats slabs: split ntiles into <=16 roughly even pieces (SBUF budget)
    nslab = min(16, ntiles)
    slab_sizes = [ntiles // nslab + (1 if i < ntiles % nslab else 0)
                  for i in range(nslab)]

    with tile.TileContext(nc) as tc:
        with tc.tile_pool(name="cst", bufs=1) as cst, \
             tc.tile_pool(name="gpool", bufs=2) as gpool, \
             tc.tile_pool(name="cpool", bufs=2) as cpool, \
             tc.tile_pool(name="gtpool", bufs=2) as gtpool, \
             tc.tile_pool(name="stat", bufs=2) as stat, \
             tc.tile_pool(name="ps", bufs=2, space="PSUM") as ps, \
             tc.tile_pool(name="ps2", bufs=2, space="PSUM") as ps2:
            w_sb = cst.tile([P, K3 * P], f32)
            nc.sync.dma_start(w_sb[:], wcat[:])
            id_sb = cst.tile([P, P], f32)
            nc.sync.dma_start(id_sb[:], ident[:])
            gidx_sb = cst.tile([P, NB * CPB], i32)
            nc.sync.dma_start(gidx_sb[:], gidx_d[:])
            sidx_sb = cst.tile([P, NB * CPB], i32)
            nc.sync.dma_start(sidx_sb[:], sidx_d[:])

            ci = 0
            for b in range(NB):
                g_st = gpool.tile([P, CPB, P], f32, tag="gst")
                for j in range(CPB):
                    col = b * CPB + j
                    nc.gpsimd.indirect_dma_start(
                        out=g_st[:, j, :], out_offset=None, in_=feats[:],
                        in_offset=IndirectOffsetOnAxis(
                            ap=gidx_sb[:, col:col + 1], axis=0))
                c_st = cpool.tile([P, CPB, P], f32, tag="cstg")
                for q in range(CPB // 4):
                    gt_ps = ps.tile([P, 4 * P], f32, tag="gtps")
                    for j4 in range(4):
                        j = q * 4 + j4
                        nc.tensor.transpose(gt_ps[:, j4 * P:(j4 + 1) * P],
                                            g_st[:, j, :], id_sb[:])
                    gt_sb = gtpool.tile([P, 4 * P], f32, tag="gtsb")
                    nc.vector.tensor_copy(gt_sb[:], gt_ps[:])
                    c_ps = ps2.tile([P, 4 * P], f32, tag="cps")
                    for j4 in range(4):
                        kk = int(chunk_k[b * CPB + q * 4 + j4])
                        nc.tensor.matmul(c_ps[:, j4 * P:(j4 + 1) * P],
                                         lhsT=gt_sb[:, j4 * P:(j4 + 1) * P],
                                         rhs=w_sb[:, kk * P:(kk + 1) * P],
                                         start=True, stop=True)
                    nc.vector.tensor_copy(c_st[:, q * 4:(q + 1) * 4, :], c_ps[:])
                for j in range(CPB):
                    col = b * CPB + j
                    nc.gpsimd.indirect_dma_start(
                        out=raw[:],
                        out_offset=IndirectOffsetOnAxis(
                            ap=sidx_sb[:, col:col + 1], axis=0),
                        in_=c_st[:, j, :],
                        in_offset=None,
                        compute_op=mybir.AluOpType.add)

            # ---- BN partial stats: sum and sum-of-squares per channel ----
            psum_t = cst.tile([P, P], f32)
            psq_t = cst.tile([P, P], f32)
            nc.gpsimd.memset(psum_t[:], 0.0)
            nc.gpsimd.memset(psq_t[:], 0.0)
            r0 = 0
            for T in slab_sizes:
                sl = stat.tile([P, T, P], f32, tag="slab")
                nc.sync.dma_start(
                    sl[:], raw[r0 * P:(r0 + T) * P, :].rearrange(
                        "(t p) c -> p t c", p=P))
                sq = stat.tile([P, T, P], f32, tag="sq")
                nc.vector.tensor_tensor(out=sq[:], in0=sl[:], in1=sl[:],
                                        op=mybir.AluOpType.mult)
                red = stat.tile([P, P], f32, tag="red")
                nc.vector.tensor_reduce(out=red[:], in_=sl[:].rearrange("p t c -> p c t"),
                                        axis=mybir.AxisListType.X,
                                        op=mybir.AluOpType.add)
                nc.vector.tensor_tensor(out=psum_t[:], in0=psum_t[:], in1=red[:],
                                        op=mybir.AluOpType.add)
                red2 = stat.tile([P, P], f32, tag="red2")
                nc.vector.tensor_reduce(out=red2[:], in_=sq[:].rearrange("p t c -> p c t"),
                                        axis=mybir.AxisListType.X,
                                        op=mybir.AluOpType.add)
                nc.vector.tensor_tensor(out=psq_t[:], in0=psq_t[:], in1=red2[:],
                                        op=mybir.AluOpType.add)
                r0 += T
            both = cst.tile([P, 2 * P], f32)
            nc.vector.tensor_copy(both[:, :P], psum_t[:])
            nc.vector.tensor_copy(both[:, P:], psq_t[:])
            ones = cst.tile([P, 1], f32)
            nc.gpsimd.memset(ones[:], 1.0)
            st_ps = ps.tile([1, 2 * P], f32, tag="stps")
            nc.tensor.matmul(st_ps[:], lhsT=ones[:], rhs=both[:],
                             start=True, stop=True)
            st_sb = cst.tile([1, 2 * P], f32)
            nc.vector.tensor_copy(st_sb[:], st_ps[:])
            nc.sync.dma_start(stats[:], st_sb[:])
    nc.compile()
    return nc


def _build_launch2(SH):
    nc = bacc.Bacc("TRN2", target_bir_lowering=False, debug=False,
                   num_devices=N_CORES)
    GARB = _garb(SH)
    raw = nc.dram_tensor("raw", [SH + GARB, P], f32, kind="ExternalInput")
    scale = nc.dram_tensor("scale", [1, P], f32, kind="ExternalInput")
    shift = nc.dram_tensor("shift", [1, P], f32, kind="ExternalInput")
    y = nc.dram_tensor("y", [SH, P], f32, kind="ExternalOutput")

    full_tiles = SH // P
    tail = SH - full_tiles * P
    nslab = min(4, max(1, full_tiles))
    slab_sizes = [full_tiles // nslab + (1 if i < full_tiles % nslab else 0)
                  for i in range(nslab)]
    with tile.TileContext(nc) as tc:
        with tc.tile_pool(name="cst", bufs=1) as cst, \
             tc.tile_pool(name="sl", bufs=2) as slp:
            sc_sb = cst.tile([P, P], f32)
            nc.sync.dma_start(sc_sb[:], scale[:].to_broadcast([P, P]))
            sh_sb = cst.tile([P, P], f32)
            nc.sync.dma_start(sh_sb[:], shift[:].to_broadcast([P, P]))
            r0 = 0
            for T in slab_sizes:
                if T == 0:
                    continue
                sl = slp.tile([P, T, P], f32, tag="slab")
                nc.sync.dma_start(
                    sl[:], raw[r0 * P:(r0 + T) * P, :].rearrange(
                        "(t p) c -> p t c", p=P))
                nc.vector.tensor_tensor(
                    out=sl[:], in0=sl[:],
                    in1=sc_sb[:, None, :].to_broadcast([P, T, P]),
                    op=mybir.AluOpType.mult)
                nc.vector.tensor_tensor(
                    out=sl[:], in0=sl[:],
                    in1=sh_sb[:, None, :].to_broadcast([P, T, P]),
                    op=mybir.AluOpType.add)
                nc.sync.dma_start(
                    y[r0 * P:(r0 + T) * P, :].rearrange("(t p) c -> p t c", p=P),
                    sl[:])
                r0 += T
            if tail:
                tl = slp.tile([P, P], f32, tag="tail")
                nc.sync.dma_start(tl[:tail, :], raw[full_tiles * P:SH, :])
                nc.vector.tensor_tensor(out=tl[:tail, :], in0=tl[:tail, :],
                                        in1=sc_sb[:tail, :],
                                        op=mybir.AluOpType.mult)
                nc.vector.tensor_tensor(out=tl[:tail, :], in0=tl[:tail, :],
                                        in1=sh_sb[:tail, :],
                                        op=mybir.AluOpType.add)
                nc.sync.dma_start(y[full_tiles * P:SH, :], tl[:tail, :])
    nc.compile()
    return nc


def kernel(feats, W, gamma, beta, in_maps, out_maps, n_out):
    feats = np.asarray(feats, np.float32)
    W = np.asarray(W, np.float32)
    gamma = np.asarray(gamma, np.float32)
    beta = np.asarray(beta, np.float32)
    in_maps = np.asarray(in_maps)
    out_maps = np.asarray(out_maps)
    n_out = int(n_out)
    n_in, C = feats.shape
    assert C == P
    K3 = W.shape[0]

    prep = _host_prep(in_maps, out_maps, n_in, n_out)
    SH, NB = prep["SH"], prep["NB"]

    feats_z = np.concatenate([feats, np.zeros((1, P), np.float32)], axis=0)
    wcat = np.ascontiguousarray(W.transpose(1, 0, 2).reshape(P, K3 * P))
    ident = np.eye(P, dtype=np.float32)

    nc1 = _build_launch1(n_in + 1, SH, K3, NB, prep["chunk_k"], prep["calls"])
    in_maps1 = [dict(feats=feats_z, wcat=wcat, ident=ident,
                     gidx=np.ascontiguousarray(prep["gidx"][c]),
                     sidx=np.ascontiguousarray(prep["sidx"][c]))
                for c in range(N_CORES)]
    _trace = os.environ.get("BASS_KERNEL_TRACE") == "1"
    LAST_EXEC_NS.clear()
    LAST_WALL_S.clear()
    _t = time.time()
    try:
        res1 = run_bass_kernel_spmd(nc1, in_maps1,
                                    core_ids=list(range(N_CORES)),
                                    trace=_trace)
    except ModuleNotFoundError:
        res1 = run_bass_kernel_spmd(nc1, in_maps1,
                                    core_ids=list(range(N_CORES)))
    LAST_WALL_S.append(time.time() - _t)
    if res1.exec_time_ns is not None:
        LAST_EXEC_NS.append(res1.exec_time_ns)
    raws = [res1.results[c]["raw"] for c in range(N_CORES)]
    stats = np.stack([res1.results[c]["stats"].reshape(2, P)
                      for c in range(N_CORES)])

    tot_sum = stats[:, 0, :].sum(axis=0)
    tot_sq = stats[:, 1, :].sum(axis=0)
    mean = tot_sum / n_out
    var = tot_sq / n_out - mean * mean
    scale = (gamma / np.sqrt(var + EPS)).astype(np.float32)
    shift = (beta - mean * scale).astype(np.float32)

    nc2 = _build_launch2(SH)
    in_maps2 = [dict(raw=raws[c], scale=scale.reshape(1, P),
                     shift=shift.reshape(1, P)) for c in range(N_CORES)]
    _t = time.time()
    try:
        res2 = run_bass_kernel_spmd(nc2, in_maps2,
                                    core_ids=list(range(N_CORES)),
                                    trace=_trace)
    except ModuleNotFoundError:
        res2 = run_bass_kernel_spmd(nc2, in_maps2,
                                    core_ids=list(range(N_CORES)))
    LAST_WALL_S.append(time.time() - _t)
    if res2.exec_time_ns is not None:
        LAST_EXEC_NS.append(res2.exec_time_ns)
    y = np.concatenate([res2.results[c]["y"] for c in range(N_CORES)], axis=0)
    return y



# revision 2
# speedup vs baseline: 6.3429x; 6.3429x over previous
"""Trainium2 Bass kernel for sparse conv-transpose (gather-GEMM-scatter) + BatchNorm.

Single-launch design, 8 NeuronCores SPMD, output-row sharded (SH=n_out/8 rows
per core), scatter-free:

  - feats are uploaded fp16 SHARDED (n_in/8 rows per core) and AllGathered
    on-device into each core's DRAM -> 8x less host->device traffic.
  - Pairs (k, m) are grouped host-side by (core, out-row-chunk of CH, k) and
    padded to 128-slot blocks (block count per (chunk,k) shared across cores =
    max over cores; pad slots get index 2^30 and are skipped by the gather's
    bounds check).
  - Per chunk: one multi-column indirect-DMA gathers all the chunk's feats
    rows into SBUF (partition = pair slot % 128).
  - Stage 1 (per block, per k): S[p, j] = (sidx[p] == j) selection matrix is
    built on DVE (one is_equal op); matmul(lhsT=G_block, rhs=S) accumulates
    C_k[cin, or] in PSUM.  This performs gather-sum + transpose in one op --
    the "scatter" becomes PSUM accumulation, no DMA scatter at all.
  - Stage 2 (per k): matmul(lhsT=W_k, rhs=C_k) accumulates O[cout, or] in
    PSUM over all k.
  - BN stats (sum, sum-of-squares per channel) accumulate on DVE from O
    chunks; a tiny AllReduce combines them across cores; scale/shift are
    computed on-device and applied channel-major; PE-transpose emits y
    row-major as fp16 (host upcasts to f32).
"""

import sys

sys.path.insert(0, "/opt/trn_rl_repo")

import os
import time

import numpy as np

import concourse.bacc as bacc
import concourse.tile as tile
from concourse import bass, mybir
from concourse.bass import IndirectOffsetOnAxis
from concourse.bass_utils import run_bass_kernel_spmd

P = 128
N_CORES = 8
EPS = 1e-5
LAST_EXEC_NS = []  # exec_time_ns per launch (when NTFF tracing is available)
LAST_WALL_S = []   # wall seconds per launch (incl. PJRT transfer)

f32 = mybir.dt.float32
fp16 = mybir.dt.float16
i32 = mybir.dt.int32
AO = mybir.AluOpType


def _host_prep(in_maps, out_maps, n_in, n_out, ch):
    """Group pairs by (core, chunk, k); pad each group to 128-slot blocks with
    a block count shared across cores. Returns slot-layout index arrays."""
    K3, M = in_maps.shape
    SH = (n_out + N_CORES - 1) // N_CORES
    NCH = (SH + ch - 1) // ch
    im = np.asarray(in_maps, np.int64).reshape(-1)
    om = np.asarray(out_maps, np.int64).reshape(-1)
    kk = np.repeat(np.arange(K3, dtype=np.int64), M)
    core = om // SH
    orl = om - core * SH
    chk = orl // ch
    orc = orl - chk * ch

    gkey = (core * NCH + chk) * K3 + kk
    cnt = np.bincount(gkey, minlength=N_CORES * NCH * K3).reshape(
        N_CORES, NCH, K3)
    nblk = np.maximum((cnt + P - 1) // P, 1).max(axis=0)  # [NCH, K3] shared
    base_blk = np.concatenate([[0], np.cumsum(nblk.reshape(-1))])
    COLS = int(base_blk[-1])

    order = np.argsort(gkey, kind="stable")
    gs = gkey[order]
    starts = np.r_[0, np.flatnonzero(gs[1:] != gs[:-1]) + 1]
    counts = np.diff(np.r_[starts, len(gs)])
    ranks = np.arange(len(gs)) - np.repeat(starts, counts)
    ck = gs % (NCH * K3)
    cor = gs // (NCH * K3)
    slot = base_blk[ck] * P + ranks

    gidx = np.full((N_CORES, COLS * P), 0, np.int32)
    sidx = np.full((N_CORES, COLS * P), -1.0, np.float16)
    gidx[cor, slot] = im[order]
    sidx[cor, slot] = orc[order].astype(np.float16)
    gidx = np.ascontiguousarray(gidx.reshape(N_CORES, COLS, P).transpose(0, 2, 1))
    sidx = np.ascontiguousarray(sidx.reshape(N_CORES, COLS, P).transpose(0, 2, 1))
    return dict(SH=SH, NCH=NCH, COLS=COLS, nblk=nblk, gidx=gidx, sidx=sidx)


def _build(n_in, SHF, SH, n_out, K3, ch, NCH, COLS, nblk, debug=False):
    nc = bacc.Bacc("TRN2", target_bir_lowering=False, debug=False,
                   num_devices=N_CORES)
    feats_s = nc.dram_tensor("feats_s", [SHF, P], fp16, kind="ExternalInput")
    wcat = nc.dram_tensor("wcat", [P, K3 * P], fp16, kind="ExternalInput")
    gidx_d = nc.dram_tensor("gidx", [P, COLS], i32, kind="ExternalInput")
    sidx_d = nc.dram_tensor("sidx", [P, COLS], fp16, kind="ExternalInput")
    iota_d = nc.dram_tensor("iota", [1, ch], f32, kind="ExternalInput")
    gb_d = nc.dram_tensor("gb", [1, 2 * P], f32, kind="ExternalInput")
    y = nc.dram_tensor("y", [SH, P], fp16, kind="ExternalOutput")

    feats_i = nc.dram_tensor("feats_i", [SHF, P], fp16, kind="Internal")
    feats_full = nc.dram_tensor("feats_full", [N_CORES * SHF, P], fp16,
                                kind="Internal", addr_space="Shared")
    SHP = NCH * ch
    raw = nc.dram_tensor("raw", [P, SHP], f32, kind="Internal")
    st_in = nc.dram_tensor("st_in", [P, 2], f32, kind="Internal")
    st_out = nc.dram_tensor("st_out", [P, 2], f32, kind="Internal",
                            addr_space="Shared")
    RG = [list(range(N_CORES))]
    if debug:
        DB = min(8, int(nblk.sum(axis=1)[0]))
        g_dump = nc.dram_tensor("g_dump", [P, DB, P], fp16, kind="ExternalOutput")
        s_dump = nc.dram_tensor("s_dump", [P, ch], fp16, kind="ExternalOutput")
        c_dump = nc.dram_tensor("c_dump", [P, ch], fp16, kind="ExternalOutput")
        o_dump = nc.dram_tensor("o_dump", [P, ch], f32, kind="ExternalOutput")
        st_dump = nc.dram_tensor("st_dump", [P, 2], f32, kind="ExternalOutput")
        id_dump = nc.dram_tensor("id_dump", [P, P], fp16, kind="ExternalOutput")
        ff_dump = nc.dram_tensor("ff_dump", [P, P], fp16, kind="ExternalOutput")

    HW = min(512, ch)  # matmul free width (one fp32 PSUM bank)
    NH = ch // HW
    BMAX = int(nblk.sum(axis=1).max())
    chunk_cols = np.concatenate([[0], np.cumsum(nblk.sum(axis=1))])

    with tile.TileContext(nc) as tc:
        with tc.tile_pool(name="cst", bufs=1) as cst:
            w_sb = cst.tile([P, K3 * P], fp16)
            nc.sync.dma_start(w_sb[:], wcat[:])
            gidx_sb = cst.tile([P, COLS], i32)
            nc.sync.dma_start(gidx_sb[:], gidx_d[:])
            sidx_h = cst.tile([P, COLS], fp16)
            nc.sync.dma_start(sidx_h[:], sidx_d[:])
            sidx_sb = cst.tile([P, COLS], f32)
            nc.vector.tensor_copy(sidx_sb[:], sidx_h[:])
            iota_sb = cst.tile([P, ch], f32)
            nc.sync.dma_start(iota_sb[:], iota_d[:].to_broadcast([P, ch]))
            pidx = cst.tile([P, 1], f32)
            nc.sync.dma_start(pidx[:], iota_d[:, :P].rearrange("a c -> c a"))
            ident = cst.tile([P, P], fp16)
            nc.vector.tensor_scalar(out=ident[:], in0=iota_sb[:, :P],
                                    scalar1=pidx[:], scalar2=None,
                                    op0=AO.is_equal)
            if debug:
                nc.sync.dma_start(id_dump[:], ident[:])
            gam = cst.tile([P, 1], f32)
            nc.sync.dma_start(gam[:], gb_d[:, :P].rearrange("a c -> c a"))
            bet = cst.tile([P, 1], f32)
            nc.sync.dma_start(bet[:], gb_d[:, P:].rearrange("a c -> c a"))
            A = cst.tile([P, ch], f32)
            nc.gpsimd.memset(A[:], 0.0)
            SQ = cst.tile([P, ch], f32)
            nc.gpsimd.memset(SQ[:], 0.0)

            nc.sync.dma_start(feats_i[:], feats_s[:])
            nc.gpsimd.collective_compute(
                "AllGather", AO.bypass, RG, ins=[feats_i[:]],
                outs=[feats_full[:]])
            if debug:
                ff_sb = cst.tile([P, P], fp16)
                nc.sync.dma_start(ff_sb[:], feats_full[:P, :])
                nc.sync.dma_start(ff_dump[:], ff_sb[:])

            with tc.tile_pool(name="gp", bufs=2) as gp, \
                 tc.tile_pool(name="sp", bufs=3) as sp, \
                 tc.tile_pool(name="csb", bufs=2) as csbp, \
                 tc.tile_pool(name="osb", bufs=2) as osbp, \
                 tc.tile_pool(name="sqp", bufs=2) as sqp, \
                 tc.tile_pool(name="cps", bufs=2, space="PSUM") as cpsp, \
                 tc.tile_pool(name="ops", bufs=2, space="PSUM") as opsp:
                # pre-zero the G ring so skipped pad slots stay finite
                for _ in range(2):
                    g0 = gp.tile([P, BMAX, P], fp16, tag="g")
                    nc.gpsimd.memset(g0[:], 0.0)
                for c in range(NCH):
                    col0 = int(chunk_cols[c])
                    Bc = int(chunk_cols[c + 1]) - col0
                    G = gp.tile([P, BMAX, P], fp16, tag="g")
                    for j in range(Bc):
                        nc.gpsimd.indirect_dma_start(
                            out=G[:, j, :], out_offset=None,
                            in_=feats_full[:],
                            in_offset=IndirectOffsetOnAxis(
                                ap=gidx_sb[:, col0 + j:col0 + j + 1], axis=0))
                    if debug and c == 0:
                        nc.sync.dma_start(g_dump[:], G[:, :DB, :])
                    Ops = opsp.tile([P, ch], f32, tag="o")
                    bcol = col0
                    for k in range(K3):
                        nb = int(nblk[c][k])
                        Cps = cpsp.tile([P, ch], f32, tag="c")
                        for b in range(nb):
                            S = sp.tile([P, ch], fp16, tag="s")
                            nc.vector.tensor_scalar(
                                out=S[:], in0=iota_sb[:],
                                scalar1=sidx_sb[:, bcol:bcol + 1],
                                scalar2=None, op0=AO.is_equal)
                            if debug and c == 0 and k == 0 and b == 0:
                                nc.sync.dma_start(s_dump[:], S[:])
                            for h in range(NH):
                                nc.tensor.matmul(
                                    Cps[:, h * HW:(h + 1) * HW],
                                    lhsT=G[:, bcol - col0, :],
                                    rhs=S[:, h * HW:(h + 1) * HW],
                                    start=(b == 0), stop=(b == nb - 1))
                            bcol += 1
                        Csb = csbp.tile([P, ch], fp16, tag="cs")
                        nc.vector.tensor_copy(Csb[:], Cps[:])
                        if debug and c == 0 and k == 0:
                            nc.sync.dma_start(c_dump[:], Csb[:])
                        for h in range(NH):
                            nc.tensor.matmul(
                                Ops[:, h * HW:(h + 1) * HW],
                                lhsT=w_sb[:, k * P:(k + 1) * P],
                                rhs=Csb[:, h * HW:(h + 1) * HW],
                                start=(k == 0), stop=(k == K3 - 1))
                    Osb = osbp.tile([P, ch], f32, tag="ob")
                    nc.vector.tensor_copy(Osb[:], Ops[:])
                    if debug and c == 0:
                        nc.sync.dma_start(o_dump[:], Osb[:])
                    nc.vector.tensor_tensor(out=A[:], in0=A[:], in1=Osb[:],
                                            op=AO.add)
                    sqt = sqp.tile([P, ch], f32, tag="sq")
                    nc.vector.tensor_tensor(out=sqt[:], in0=Osb[:],
                                            in1=Osb[:], op=AO.mult)
                    nc.vector.tensor_tensor(out=SQ[:], in0=SQ[:], in1=sqt[:],
                                            op=AO.add)
                    nc.sync.dma_start(raw[:, c * ch:(c + 1) * ch], Osb[:])

            # ---- BN stats -> allreduce -> scale/shift ----
            st_sb = cst.tile([P, 2], f32)
            nc.vector.tensor_reduce(out=st_sb[:, 0:1], in_=A[:],
                                    axis=mybir.AxisListType.X, op=AO.add)
            nc.vector.tensor_reduce(out=st_sb[:, 1:2], in_=SQ[:],
                                    axis=mybir.AxisListType.X, op=AO.add)
            nc.sync.dma_start(st_in[:], st_sb[:])
            nc.gpsimd.collective_compute(
                "AllReduce", AO.add, RG, ins=[st_in[:]], outs=[st_out[:]])
            st2 = cst.tile([P, 2], f32)
            nc.sync.dma_start(st2[:], st_out[:])
            if debug:
                nc.sync.dma_start(st_dump[:], st2[:])
            mean = cst.tile([P, 1], f32)
            nc.vector.tensor_scalar(out=mean[:], in0=st2[:, 0:1],
                                    scalar1=1.0 / n_out, scalar2=None,
                                    op0=AO.mult)
            var = cst.tile([P, 1], f32)
            nc.vector.tensor_tensor(out=var[:], in0=mean[:], in1=mean[:],
                                    op=AO.mult)
            esq = cst.tile([P, 1], f32)
            nc.vector.tensor_scalar(out=esq[:], in0=st2[:, 1:2],
                                    scalar1=1.0 / n_out, scalar2=None,
                                    op0=AO.mult)
            nc.vector.tensor_tensor(out=var[:], in0=esq[:], in1=var[:],
                                    op=AO.subtract)
            nc.vector.tensor_scalar(out=var[:], in0=var[:], scalar1=EPS,
                                    scalar2=None, op0=AO.add)
            std = cst.tile([P, 1], f32)
            nc.scalar.activation(std[:], var[:],
                                 mybir.ActivationFunctionType.Sqrt)
            rstd = cst.tile([P, 1], f32)
            nc.vector.reciprocal(rstd[:], std[:])
            scl = cst.tile([P, 1], f32)
            nc.vector.tensor_tensor(out=scl[:], in0=gam[:], in1=rstd[:],
                                    op=AO.mult)
            shf = cst.tile([P, 1], f32)
            nc.vector.tensor_tensor(out=shf[:], in0=mean[:], in1=scl[:],
                                    op=AO.mult)
            nc.vector.tensor_tensor(out=shf[:], in0=bet[:], in1=shf[:],
                                    op=AO.subtract)

            # ---- apply + transpose to row-major y (fp16) ----
            with tc.tile_pool(name="ap", bufs=3) as app, \
                 tc.tile_pool(name="yp", bufs=2) as yp, \
                 tc.tile_pool(name="tp", bufs=2, space="PSUM") as tpp:
                for s in range(NCH):
                    r0 = s * ch
                    rows = min(SH - r0, ch)
                    if rows <= 0:
                        break
                    t = app.tile([P, ch], f32, tag="t")
                    nc.sync.dma_start(t[:], raw[:, s * ch:(s + 1) * ch])
                    tb = app.tile([P, ch], fp16, tag="tb")
                    nc.vector.tensor_scalar(out=tb[:], in0=t[:],
                                            scalar1=scl[:], scalar2=shf[:],
                                            op0=AO.mult, op1=AO.add)
                    ysb = yp.tile([P, ch // P, P], fp16, tag="y")
                    ngrp = (min(rows + P - 1, ch) + P - 1) // P
                    ngrp = (rows + P - 1) // P
                    for q in range((ngrp + 3) // 4):
                        j0 = q * 4
                        jn = min(4, ngrp - j0)
                        ps = tpp.tile([P, 4 * P], fp16, tag="tp")
                        for j in range(jn):
                            g = j0 + j
                            nc.tensor.transpose(ps[:, j * P:(j + 1) * P],
                                                tb[:, g * P:(g + 1) * P],
                                                ident[:])
                        nc.vector.tensor_copy(
                            ysb[:, j0:j0 + jn, :], ps[:, :jn * P])
                    nfull = rows // P
                    if nfull:
                        nc.sync.dma_start(
                            y[r0:r0 + nfull * P, :].rearrange(
                                "(g p) c -> p g c", p=P),
                            ysb[:, :nfull, :])
                    rem = rows - nfull * P
                    if rem:
                        nc.sync.dma_start(y[r0 + nfull * P:r0 + rows, :],
                                          ysb[:rem, nfull, :])
    nc.compile()
    return nc


def kernel(feats, W, gamma, beta, in_maps, out_maps, n_out, ch=1024):
    feats = np.asarray(feats, np.float32)
    W = np.asarray(W, np.float32)
    gamma = np.asarray(gamma, np.float32)
    beta = np.asarray(beta, np.float32)
    in_maps = np.asarray(in_maps)
    out_maps = np.asarray(out_maps)
    n_out = int(n_out)
    n_in, C = feats.shape
    assert C == P
    K3 = W.shape[0]

    prep = _host_prep(in_maps, out_maps, n_in, n_out, ch)
    SH, NCH, COLS, nblk = prep["SH"], prep["NCH"], prep["COLS"], prep["nblk"]

    SHF = (n_in + N_CORES - 1) // N_CORES
    feats16 = feats.astype(np.float16)
    if SHF * N_CORES > n_in:
        feats16 = np.concatenate(
            [feats16, np.zeros((SHF * N_CORES - n_in, P), np.float16)], axis=0)
    wcat = np.ascontiguousarray(
        W.transpose(1, 0, 2).reshape(P, K3 * P)).astype(np.float16)
    iota = np.arange(ch, dtype=np.float32).reshape(1, ch)
    gb = np.concatenate([gamma, beta]).reshape(1, 2 * P)

    nc = _build(n_in, SHF, SH, n_out, K3, ch, NCH, COLS, nblk)
    in_maps_l = [dict(feats_s=feats16[c * SHF:(c + 1) * SHF],
                      wcat=wcat,
                      gidx=np.ascontiguousarray(prep["gidx"][c]),
                      sidx=np.ascontiguousarray(prep["sidx"][c]),
                      iota=iota, gb=gb)
                 for c in range(N_CORES)]
    _trace = os.environ.get("BASS_KERNEL_TRACE") == "1"
    LAST_EXEC_NS.clear()
    LAST_WALL_S.clear()
    _t = time.time()
    try:
        res = run_bass_kernel_spmd(nc, in_maps_l,
                                   core_ids=list(range(N_CORES)),
                                   trace=_trace)
    except ModuleNotFoundError:
        res = run_bass_kernel_spmd(nc, in_maps_l,
                                   core_ids=list(range(N_CORES)))
    LAST_WALL_S.append(time.time() - _t)
    if res.exec_time_ns is not None:
        LAST_EXEC_NS.append(res.exec_time_ns)
    y = np.concatenate([res.results[c]["y"] for c in range(N_CORES)],
                       axis=0)[:n_out]
    return y.astype(np.float32)
